# revision 1
# baseline (speedup 1.0000x reference)
"""TRN2 Bass kernel for nn_SynthesisLayer (StyleGAN-style modulated 3D conv).

Math: for each sample b
  styles = w[b] @ affine_weight.T / sqrt(512) + affine_bias          [Cin]
  wmod   = weight * styles[None,:,None]                              [Co,Ci,27]
  dcoef  = rsqrt(sum_{ci,k} wmod^2 + 1e-8)                           [Co]
  y      = dcoef * conv3d(x[b], wmod, pad=1) + noise_const*ns + bias
  out    = clip(lrelu(y)*sqrt(2), -256, 256)

Device implementation (per core):
  - conv3d = 27 shifted matmuls (contraction over Cin=128 on partitions)
    accumulated in PSUM, weights modulated on device by styles.
  - demod + noise fold into a per-partition scale (ACT Prelu) and a K=1
    "noise matmul" with lhsT = ns/dcoef (so demod scaling recovers ns).
  - matmuls run in float32r (full PE rate, ~1e-4 rel precision).

Sharding: 8 cores = 4 samples x 2 D-halves. Each core gets a zero-padded
input slab [128, NSLAB] (33-wide rows, 33-row slices, one-slice D halo),
computes output [128, 16*32*32], host reassembles. No collectives.
"""

import math
import os
import sys

for _p in ("/opt/trn_rl_repo", "/root/.axon_site/_ro/trn_rl_repo"):
    if os.path.isdir(_p) and _p not in sys.path:
        sys.path.insert(0, _p)

import numpy as np

import concourse.mybir as mybir
from concourse import bacc
from concourse.tile import TileContext
from concourse.bass_utils import run_bass_kernel_spmd

P = 128          # Cin = Cout = 128
TAPS = 27        # 3x3x3
RES = 32
B = 4
W_DIM = 512
ROW = 33         # padded row width  (32 real + 1 zero)
SLICE = ROW * ROW  # 1089 padded slice (32 real rows + 1 zero row)
LEAD = 34        # leading zero guard (one row + one elem)
NSLICES = 18     # 16 output slices + 1 halo each side
BODY = NSLICES * SLICE
NSLAB = LEAD + BODY + 46   # tail guard; max AP end = 19637
DHALF = 16                 # output D slices per core
NOUT = DHALF * RES * RES   # 16384
DCH = 4                    # output D slices per pipelined input chunk
NSLABC = LEAD + (DCH + 2) * SLICE + 46  # 6614: chunk tile incl. halo+guards
NCHUNK = 512               # psum tile free size (one PSUM bank of fp32)
LRELU_ALPHA = 0.2
LRELU_GAIN = math.sqrt(2.0)
CLAMP = 256.0

f32 = mybir.dt.float32
f32r = mybir.dt.float32r
AF = mybir.ActivationFunctionType

_NC_CACHE = None
LAST_EXEC_NS = None


def build_nc():
    nc = bacc.Bacc("TRN2", target_bir_lowering=False, debug=False, num_devices=8)

    xs = nc.dram_tensor("xs", [P, NSLAB], f32r, kind="ExternalInput")
    wt = nc.dram_tensor("wt", [P, TAPS, P], f32r, kind="ExternalInput")
    aff = nc.dram_tensor("aff", [P, 4, P], f32r, kind="ExternalInput")
    wv = nc.dram_tensor("wv", [P, 5], f32r, kind="ExternalInput")
    ab = nc.dram_tensor("ab", [P, 1], f32, kind="ExternalInput")
    bb = nc.dram_tensor("bb", [P, 1], f32, kind="ExternalInput")
    nz = nc.dram_tensor("nz", [1, NOUT], f32, kind="ExternalInput")
    nsb = nc.dram_tensor("nsb", [P, 1], f32, kind="ExternalInput")
    ones = nc.dram_tensor("ones", [P, 2], f32r, kind="ExternalInput")
    acol = nc.dram_tensor("acol", [P, 1], f32, kind="ExternalInput")
    epsc = nc.dram_tensor("epsc", [P, 1], f32, kind="ExternalInput")
    zc = nc.dram_tensor("zc", [P, 1], f32, kind="ExternalInput")
    y = nc.dram_tensor("y", [P, NOUT], f32, kind="ExternalOutput")

    with TileContext(nc) as tc:
        with (
            tc.tile_pool(name="big", bufs=1) as big,
            tc.tile_pool(name="small", bufs=1) as small,
            tc.tile_pool(name="nzp", bufs=4) as nzp,
            tc.tile_pool(name="xchunk", bufs=2) as xchunk,
            tc.tile_pool(name="outp", bufs=4) as outp,
            tc.tile_pool(name="cpsum", bufs=6, space="PSUM") as cpsum,
            tc.tile_pool(name="spsum", bufs=1, space="PSUM") as spsum,
        ):
            # ---- loads ----
            wt_sb = big.tile([P, TAPS, P], f32r)
            nc.sync.dma_start(wt_sb[:], wt[:])
            aff_sb = small.tile([P, 4, P], f32r)
            nc.sync.dma_start(aff_sb[:], aff[:])
            wv_sb = small.tile([P, 5], f32r)
            nc.sync.dma_start(wv_sb[:], wv[:])
            ab_sb = small.tile([P, 1], f32)
            nc.sync.dma_start(ab_sb[:], ab[:])
            bb_sb = small.tile([P, 1], f32)
            nc.sync.dma_start(bb_sb[:], bb[:])
            nsb_sb = small.tile([P, 1], f32)
            nc.sync.dma_start(nsb_sb[:], nsb[:])
            ones_sb = small.tile([P, 2], f32r)
            nc.sync.dma_start(ones_sb[:], ones[:])
            acol_sb = small.tile([P, 1], f32)
            nc.sync.dma_start(acol_sb[:], acol[:])
            epsc_sb = small.tile([P, 1], f32)
            nc.sync.dma_start(epsc_sb[:], epsc[:])
            zc_sb = small.tile([P, 1], f32)
            nc.sync.dma_start(zc_sb[:], zc[:])

            # ---- styles = w[b] @ aff.T / sqrt(512) + affine_bias ----
            # N=1 fp32r matmuls are ISA-illegal; use N=2 and read col 0
            st_ps = spsum.tile([P, 2], f32, tag="st")
            for j in range(4):
                nc.tensor.matmul(
                    st_ps[:], aff_sb[:, j, :], wv_sb[:, j : j + 2],
                    start=(j == 0), stop=(j == 3),
                )
            styles = small.tile([P, 1], f32)
            nc.scalar.activation(
                styles[:], st_ps[:, :1], AF.Identity,
                bias=ab_sb[:], scale=1.0 / math.sqrt(W_DIM),
            )

            # ---- modulated weights (lhsT for the conv) ----
            wm_sb = big.tile([P, TAPS, P], f32r)
            nc.vector.tensor_scalar_mul(wm_sb[:], wt_sb[:], styles[:])

            def wm_tap(k):
                return wm_sb[:, k, :]

            # ---- demod sums: v[co] = sum_ci styles^2 * (sum_k wt^2) ----
            # (sum_k wt^2 depends only on the weight DMA, so it runs off the
            # styles critical path)
            sq_sb = big.tile([P, P, TAPS], f32)  # [ci, co, k]
            nc.vector.tensor_tensor(
                sq_sb.rearrange("p co k -> p k co"), wt_sb[:], wt_sb[:],
                mybir.AluOpType.mult,
            )
            wsq_f = big.tile([P, P], f32)  # [ci, co] = sum_k wt^2
            nc.vector.reduce_sum(wsq_f[:], sq_sb[:], axis=mybir.AxisListType.X)
            s2 = small.tile([P, 1], f32)
            nc.vector.tensor_tensor(s2[:], styles[:], styles[:], mybir.AluOpType.mult)
            w2s_f = big.tile([P, P], f32)
            nc.vector.tensor_scalar_mul(w2s_f[:], wsq_f[:], s2[:])
            w2s = big.tile([P, P], f32r)
            nc.vector.tensor_copy(w2s[:], w2s_f[:])

            vcol_ps = spsum.tile([P, 2], f32, tag="vc")
            nc.tensor.matmul(vcol_ps[:], w2s[:], ones_sb[:], start=True, stop=True)

            # S_col = sqrt(2) * rsqrt(v + 1e-8)  (per-partition ACT scale)
            veps = small.tile([P, 1], f32)
            nc.scalar.activation(veps[:], vcol_ps[:, :1], AF.Identity, bias=epsc_sb[:])
            vrec = small.tile([P, 1], f32)
            nc.vector.reciprocal(vrec[:], veps[:])
            s_col = small.tile([P, 1], f32)
            nc.scalar.activation(
                s_col[:], vrec[:], AF.Sqrt, bias=zc_sb[:], scale=LRELU_GAIN**2
            )
            # B_col = bias * sqrt(2)
            b_col = small.tile([P, 1], f32)
            nc.vector.tensor_scalar_mul(b_col[:], bb_sb[:], LRELU_GAIN)
            # noise gain = noise_strength * sqrt(2), per partition
            nsg = small.tile([P, 1], f32)
            nc.vector.tensor_scalar_mul(nsg[:], nsb_sb[:], LRELU_GAIN)

            # ---- main conv loop: variable input chunks (double-buffered);
            # the first chunk is small so PE starts sooner ----
            CHUNKS = [(1, 1), (2, 3), (5, 4), (9, 4), (13, 4)]  # (a, n_out)
            for a, n in CHUNKS:
                xt = xchunk.tile([P, NSLABC], f32r, tag="xchunk")
                s_g = (a - 1) * SLICE  # chunk start in the padded DRAM slab
                wlen = LEAD + (n + 2) * SLICE + 46
                step = 2 * SLICE
                for i0 in range(0, wlen, step):
                    bnd = min(wlen, i0 + step)
                    nc.sync.dma_start(xt[:, i0:bnd], xs[:, s_g + i0 : s_g + bnd])
                for dl in range(1, n + 1):       # local padded slice index
                    d = a + dl - 1               # global padded slice index
                    for half in range(2):        # 16 rows each
                        n0 = LEAD + dl * SLICE + half * 16 * ROW
                        pt = cpsum.tile([P, NCHUNK], f32, tag="conv")
                        for k in range(TAPS):
                            kd, r = divmod(k, 9)
                            kh, kw = divmod(r, 3)
                            q = n0 + (kd - 1) * SLICE + (kh - 1) * ROW + (kw - 1)
                            rhs = xt[:, q : q + 528].rearrange(
                                "p (r c) -> p r c", c=ROW
                            )[:, :, :RES]
                            nc.tensor.matmul(
                                pt[:], wm_tap(k), rhs,
                                start=(k == 0), stop=(k == TAPS - 1),
                            )
                        off = (d - 1) * 1024 + half * NCHUNK
                        # noise: DMA-broadcast the chunk to all partitions,
                        # scale by ns*sqrt(2) on DVE (PE stays conv-only)
                        nz_bc = nzp.tile([P, 1, NCHUNK], f32, tag="nz")
                        nc.sync.dma_start(
                            nz_bc[:], nz[:, off : off + NCHUNK].partition_broadcast(P)
                        )
                        nc.vector.tensor_scalar_mul(nz_bc[:], nz_bc[:], nsg[:])

                        ut = outp.tile([P, NCHUNK], f32, tag="out")
                        # ut = psum * (dcoef*sqrt2) + noise_term
                        nc.vector.scalar_tensor_tensor(
                            ut[:], pt[:], s_col[:], nz_bc[:, 0, :],
                            mybir.AluOpType.mult, mybir.AluOpType.add,
                        )
                        nc.scalar.activation(
                            ut[:], ut[:], AF.Prelu,
                            bias=b_col[:], scale=1.0, alpha=acol_sb[:],
                        )
                        nc.vector.tensor_scalar(
                            ut[:], ut[:], CLAMP, -CLAMP,
                            mybir.AluOpType.min, mybir.AluOpType.max,
                        )
                        nc.sync.dma_start(y[:, off : off + NCHUNK], ut[:])

    nc.compile()
    return nc


def _get_nc():
    global _NC_CACHE
    if _NC_CACHE is None:
        _NC_CACHE = build_nc()
    return _NC_CACHE


def _make_core_inputs(x, w, affine_weight, affine_bias, weight, noise_const,
                      noise_strength, bias):
    """Build the 8 per-core input maps (host-side sharding / layout only)."""
    aff_host = np.ascontiguousarray(
        affine_weight.T.reshape(4, P, P).transpose(1, 0, 2)
    )  # [wd_p, j, ci]
    wt_host = np.ascontiguousarray(
        weight.reshape(P, P, TAPS).transpose(1, 2, 0)
    )  # [ci, k, co]
    ab_host = affine_bias.reshape(P, 1).astype(np.float32)
    bb_host = bias.reshape(P, 1).astype(np.float32)
    nsb_host = np.full((P, 1), float(noise_strength.reshape(-1)[0]), np.float32)
    ones_host = np.ones((P, 2), np.float32)
    acol_host = np.full((P, 1), LRELU_ALPHA, np.float32)
    epsc_host = np.full((P, 1), 1e-8, np.float32)
    zc_host = np.zeros((P, 1), np.float32)

    in_maps = []
    for c in range(8):
        b, half = divmod(c, 2)
        d0 = DHALF * half
        slab = np.zeros((P, NSLAB), np.float32)
        view = slab[:, LEAD : LEAD + BODY].reshape(P, NSLICES, ROW, ROW)
        lo = max(0, d0 - 1)
        hi = min(RES, d0 + DHALF + 1)
        # padded slice s holds global slice d0-1+s
        view[:, lo - (d0 - 1) : hi - (d0 - 1), :RES, :RES] = x[b, :, lo:hi]
        nz_host = np.ascontiguousarray(
            noise_const[d0 : d0 + DHALF].reshape(1, NOUT)
        )
        wv_host = np.zeros((P, 5), np.float32)
        wv_host[:, :4] = w[b].reshape(4, P).T
        in_maps.append({
            "xs": slab,
            "wt": wt_host,
            "aff": aff_host,
            "wv": wv_host,
            "ab": ab_host,
            "bb": bb_host,
            "nz": nz_host,
            "nsb": nsb_host,
            "ones": ones_host,
            "acol": acol_host,
            "epsc": epsc_host,
            "zc": zc_host,
        })
    return in_maps


def kernel(x, w, affine_weight, affine_bias, weight, noise_const,
           noise_strength, bias):
    global LAST_EXEC_NS
    x = np.asarray(x, np.float32)
    w = np.asarray(w, np.float32)
    affine_weight = np.asarray(affine_weight, np.float32)
    affine_bias = np.asarray(affine_bias, np.float32)
    weight = np.asarray(weight, np.float32)
    noise_const = np.asarray(noise_const, np.float32)
    noise_strength = np.asarray(noise_strength, np.float32)
    bias = np.asarray(bias, np.float32)

    nc = _get_nc()
    in_maps = _make_core_inputs(
        x, w, affine_weight, affine_bias, weight, noise_const,
        noise_strength, bias,
    )
    trace = bool(os.environ.get("KERNEL_TRACE"))
    if trace:
        from concourse.bass_utils import axon_active

        if axon_active():
            try:  # axon NTFF capture needs the profile hook; absent in some pods
                from antenv.axon_hooks import get_axon_ntff_profile_hook  # noqa: F401
            except ImportError:
                trace = False
    res = run_bass_kernel_spmd(nc, in_maps, core_ids=list(range(8)), trace=trace)
    LAST_EXEC_NS = res.exec_time_ns

    out = np.empty((B, P, RES, RES, RES), np.float32)
    for c in range(8):
        b, half = divmod(c, 2)
        d0 = DHALF * half
        out[b, :, d0 : d0 + DHALF] = res.results[c]["y"].reshape(
            P, DHALF, RES, RES
        )
    return out



# revision 2
# speedup vs baseline: 1.2889x; 1.2889x over previous
"""TRN2 Bass kernel for nn_SynthesisLayer (StyleGAN-style modulated 3D conv).

Math: for each sample b
  styles = w[b] @ affine_weight.T / sqrt(512) + affine_bias          [Cin]
  wmod   = weight * styles[None,:,None]                              [Co,Ci,27]
  dcoef  = rsqrt(sum_{ci,k} wmod^2 + 1e-8)                           [Co]
  y      = dcoef * conv3d(x[b], wmod, pad=1) + noise_const*ns + bias
  out    = clip(lrelu(y)*sqrt(2), -256, 256)

Device implementation (per core):
  - conv3d = shifted matmuls (contraction over Cin=128 on partitions)
    accumulated in PSUM, weights modulated on device by styles.
  - fp8 DoubleRow path: x is pre-split on the host into e4m3 hi+lo slabs;
    modulated weights are split on device into e4m3 hi+lo.  Each DoubleRow
    matmul computes two (weight, shifted-x) products per PSUM row at 0.5
    cycles/row, so the 80 products per output element (27 hi*hi + 53
    hi/lo cross terms; the w_lo correction of the last tap is dropped)
    cost 40 matmuls per 512-wide tile vs 27 full-rate fp32r matmuls.
  - demod + noise fold into a per-partition scale (ACT Prelu) and a DVE
    scalar_tensor_tensor; demod is computed exactly from the fp32 weights.

Sharding: 8 cores = 4 samples x 2 D-halves. Each core gets zero-padded
fp8 hi/lo input slabs [128, NSLAB] (33-wide rows, 33-row slices, one-slice
D halo), computes output [128, 16*32*32], host reassembles. No collectives.
"""

import math
import os
import sys

for _p in ("/opt/trn_rl_repo", "/root/.axon_site/_ro/trn_rl_repo"):
    if os.path.isdir(_p) and _p not in sys.path:
        sys.path.insert(0, _p)

import numpy as np
import ml_dtypes

import concourse.mybir as mybir
from concourse import bacc
from concourse.ap import AP
from concourse.tile import TileContext
from concourse.bass_utils import run_bass_kernel_spmd

P = 128          # Cin = Cout = 128
TAPS = 27        # 3x3x3
RES = 32
B = 4
W_DIM = 512
ROW = 33         # padded row width  (32 real + 1 zero)
SLICE = ROW * ROW  # 1089 padded slice (32 real rows + 1 zero row)
LEAD = 34        # leading zero guard (one row + one elem)
NSLICES = 18     # 16 output slices + 1 halo each side
BODY = NSLICES * SLICE
NSLAB = LEAD + BODY + 46   # tail guard; max AP end = 19637
DHALF = 16                 # output D slices per core
NOUT = DHALF * RES * RES   # 16384
DCH = 4                    # output D slices per pipelined input chunk
NSLABC = LEAD + (DCH + 2) * SLICE + 46  # 6614: chunk tile incl. halo+guards
NCHUNK = 512               # psum tile free size (one PSUM bank of fp32)
LRELU_ALPHA = 0.2
LRELU_GAIN = math.sqrt(2.0)
CLAMP = 256.0

f32 = mybir.dt.float32
f32r = mybir.dt.float32r
fp8 = mybir.dt.float8e4
DRMODE = mybir.MatmulPerfMode.DoubleRow
AF = mybir.ActivationFunctionType
E4 = ml_dtypes.float8_e4m3fn

# tap k = kd*9 + kh*3 + kw; shift of tap k relative to the tile center
TAP_OFF = [
    (kd - 1) * SLICE + (kh - 1) * ROW + (kw - 1)
    for kd in range(3) for kh in range(3) for kw in range(3)
]

_NC_CACHE = None
LAST_EXEC_NS = None


def _pair_ap(flat_ap, off, delta, inner_dims):
    """[[p],[delta,2],*inner_dims] AP at element offset `off` of a 2D AP."""
    dims = [list(flat_ap.ap[0]), [delta, 2]] + [list(d) for d in inner_dims]
    return AP(flat_ap.tensor, flat_ap.offset + off, dims)


def build_nc():
    nc = bacc.Bacc("TRN2", target_bir_lowering=False, debug=False, num_devices=8)

    xhi = nc.dram_tensor("xhi", [P, NSLAB], fp8, kind="ExternalInput")
    xlo = nc.dram_tensor("xlo", [P, NSLAB], fp8, kind="ExternalInput")
    wt = nc.dram_tensor("wt", [P, TAPS, P], f32, kind="ExternalInput")
    aff = nc.dram_tensor("aff", [P, 4, P], f32r, kind="ExternalInput")
    wv = nc.dram_tensor("wv", [P, 5], f32r, kind="ExternalInput")
    ab = nc.dram_tensor("ab", [P, 1], f32, kind="ExternalInput")
    bb = nc.dram_tensor("bb", [P, 1], f32, kind="ExternalInput")
    nz = nc.dram_tensor("nz", [1, NOUT], f32, kind="ExternalInput")
    nsb = nc.dram_tensor("nsb", [P, 1], f32, kind="ExternalInput")
    ones = nc.dram_tensor("ones", [P, 2], f32r, kind="ExternalInput")
    acol = nc.dram_tensor("acol", [P, 1], f32, kind="ExternalInput")
    epsc = nc.dram_tensor("epsc", [P, 1], f32, kind="ExternalInput")
    zc = nc.dram_tensor("zc", [P, 1], f32, kind="ExternalInput")
    y = nc.dram_tensor("y", [P, NOUT], f32, kind="ExternalOutput")

    with TileContext(nc) as tc:
        with (
            tc.tile_pool(name="big", bufs=1) as big,
            tc.tile_pool(name="small", bufs=1) as small,
            tc.tile_pool(name="nzp", bufs=4) as nzp,
            tc.tile_pool(name="xchunk", bufs=2) as xchunk,
            tc.tile_pool(name="outp", bufs=4) as outp,
            tc.tile_pool(name="cpsum", bufs=6, space="PSUM") as cpsum,
            tc.tile_pool(name="spsum", bufs=1, space="PSUM") as spsum,
        ):
            # ---- loads ----
            wt_sb = big.tile([P, TAPS, P], f32)
            nc.sync.dma_start(wt_sb[:], wt[:])
            aff_sb = small.tile([P, 4, P], f32r)
            nc.sync.dma_start(aff_sb[:], aff[:])
            wv_sb = small.tile([P, 5], f32r)
            nc.sync.dma_start(wv_sb[:], wv[:])
            ab_sb = small.tile([P, 1], f32)
            nc.sync.dma_start(ab_sb[:], ab[:])
            bb_sb = small.tile([P, 1], f32)
            nc.sync.dma_start(bb_sb[:], bb[:])
            nsb_sb = small.tile([P, 1], f32)
            nc.sync.dma_start(nsb_sb[:], nsb[:])
            ones_sb = small.tile([P, 2], f32r)
            nc.sync.dma_start(ones_sb[:], ones[:])
            acol_sb = small.tile([P, 1], f32)
            nc.sync.dma_start(acol_sb[:], acol[:])
            epsc_sb = small.tile([P, 1], f32)
            nc.sync.dma_start(epsc_sb[:], epsc[:])
            zc_sb = small.tile([P, 1], f32)
            nc.sync.dma_start(zc_sb[:], zc[:])

            # ---- styles = w[b] @ aff.T / sqrt(512) + affine_bias ----
            # N=1 fp32r matmuls are ISA-illegal; use N=2 and read col 0
            st_ps = spsum.tile([P, 2], f32, tag="st")
            for j in range(4):
                nc.tensor.matmul(
                    st_ps[:], aff_sb[:, j, :], wv_sb[:, j : j + 2],
                    start=(j == 0), stop=(j == 3),
                )
            styles = small.tile([P, 1], f32)
            nc.scalar.activation(
                styles[:], st_ps[:, :1], AF.Identity,
                bias=ab_sb[:], scale=1.0 / math.sqrt(W_DIM),
            )

            # ---- modulated weights, split into fp8 hi + lo ----
            # wq slots 0..26 = hi taps, 27..53 = lo taps
            wq = big.tile([P, 2 * TAPS, P], fp8)
            nc.vector.tensor_scalar_mul(wq[:, :TAPS, :], wt_sb[:], styles[:])
            nc.vector.scalar_tensor_tensor(
                wq[:, TAPS:, :], wt_sb[:], styles[:], wq[:, :TAPS, :],
                mybir.AluOpType.mult, mybir.AluOpType.subtract,
            )
            wq_flat = wq[:].rearrange("p a b -> p (a b)")

            # ---- demod sums: v[co] = sum_ci styles^2 * (sum_k wt^2) ----
            # (sum_k wt^2 depends only on the weight DMA, so it runs off the
            # styles critical path)
            sq_sb = big.tile([P, P, TAPS], f32)  # [ci, co, k]
            nc.vector.tensor_tensor(
                sq_sb.rearrange("p co k -> p k co"), wt_sb[:], wt_sb[:],
                mybir.AluOpType.mult,
            )
            wsq_f = big.tile([P, P], f32)  # [ci, co] = sum_k wt^2
            nc.vector.reduce_sum(wsq_f[:], sq_sb[:], axis=mybir.AxisListType.X)
            s2 = small.tile([P, 1], f32)
            nc.vector.tensor_tensor(s2[:], styles[:], styles[:], mybir.AluOpType.mult)
            w2s_f = big.tile([P, P], f32)
            nc.vector.tensor_scalar_mul(w2s_f[:], wsq_f[:], s2[:])
            w2s = big.tile([P, P], f32r)
            nc.vector.tensor_copy(w2s[:], w2s_f[:])

            vcol_ps = spsum.tile([P, 2], f32, tag="vc")
            nc.tensor.matmul(vcol_ps[:], w2s[:], ones_sb[:], start=True, stop=True)

            # S_col = sqrt(2) * rsqrt(v + 1e-8)  (per-partition ACT scale)
            veps = small.tile([P, 1], f32)
            nc.scalar.activation(veps[:], vcol_ps[:, :1], AF.Identity, bias=epsc_sb[:])
            vrec = small.tile([P, 1], f32)
            nc.vector.reciprocal(vrec[:], veps[:])
            s_col = small.tile([P, 1], f32)
            nc.scalar.activation(
                s_col[:], vrec[:], AF.Sqrt, bias=zc_sb[:], scale=LRELU_GAIN**2
            )
            # B_col = bias * sqrt(2)
            b_col = small.tile([P, 1], f32)
            nc.vector.tensor_scalar_mul(b_col[:], bb_sb[:], LRELU_GAIN)
            # noise gain = noise_strength * sqrt(2), per partition
            nsg = small.tile([P, 1], f32)
            nc.vector.tensor_scalar_mul(nsg[:], nsb_sb[:], LRELU_GAIN)

            # ---- main conv loop: variable input chunks (double-buffered);
            # the first chunk is small so PE starts sooner ----
            CHUNKS = [(1, 1), (2, 3), (5, 4), (9, 4), (13, 4)]  # (a, n_out)
            for a, n in CHUNKS:
                xt = xchunk.tile([P, 2, NSLABC], fp8, tag="xchunk")
                s_g = (a - 1) * SLICE  # chunk start in the padded DRAM slab
                wlen = LEAD + (n + 2) * SLICE + 46
                step = 2 * SLICE
                for i0 in range(0, wlen, step):
                    bnd = min(wlen, i0 + step)
                    nc.sync.dma_start(xt[:, 0, i0:bnd], xlo[:, s_g + i0 : s_g + bnd])
                    nc.sync.dma_start(xt[:, 1, i0:bnd], xhi[:, s_g + i0 : s_g + bnd])
                xt_flat = xt[:].rearrange("p a b -> p (a b)")
                for dl in range(1, n + 1):       # local padded slice index
                    d = a + dl - 1               # global padded slice index
                    for half in range(2):        # 16 rows each
                        n0 = LEAD + dl * SLICE + half * 16 * ROW
                        pt = cpsum.tile([P, NCHUNK], f32, tag="conv")
                        inner = ([ROW, 16], [1, RES])
                        # 13 hi-hi tap pairs + tap26 hi*(hi+lo), then 26
                        # hi/lo correction pairs (needs w_lo, off the
                        # startup critical path)
                        for i in range(13):
                            q0 = n0 + TAP_OFF[2 * i]
                            dq = TAP_OFF[2 * i + 1] - TAP_OFF[2 * i]
                            nc.tensor.matmul(
                                pt[:], wq[:, 2 * i : 2 * i + 2, :],
                                _pair_ap(xt_flat, NSLABC + q0, dq, inner),
                                start=(i == 0), stop=False, perf_mode=DRMODE,
                            )
                        q26 = n0 + TAP_OFF[26]
                        nc.tensor.matmul(
                            pt[:], _pair_ap(wq_flat, 26 * P, 0, ([1, P],)),
                            _pair_ap(xt_flat, q26, NSLABC, inner),
                            start=False, stop=False, perf_mode=DRMODE,
                        )
                        for k in range(26):
                            qk = n0 + TAP_OFF[k]
                            nc.tensor.matmul(
                                pt[:], _pair_ap(wq_flat, k * P, TAPS * P, ([1, P],)),
                                _pair_ap(xt_flat, qk, NSLABC, inner),
                                start=False, stop=(k == 25), perf_mode=DRMODE,
                            )
                        off = (d - 1) * 1024 + half * NCHUNK
                        # noise: DMA-broadcast the chunk to all partitions,
                        # scale by ns*sqrt(2) on DVE (PE stays conv-only)
                        nz_bc = nzp.tile([P, 1, NCHUNK], f32, tag="nz")
                        nc.sync.dma_start(
                            nz_bc[:], nz[:, off : off + NCHUNK].partition_broadcast(P)
                        )
                        nc.vector.tensor_scalar_mul(nz_bc[:], nz_bc[:], nsg[:])

                        ut = outp.tile([P, NCHUNK], f32, tag="out")
                        # ut = psum * (dcoef*sqrt2) + noise_term
                        nc.vector.scalar_tensor_tensor(
                            ut[:], pt[:], s_col[:], nz_bc[:, 0, :],
                            mybir.AluOpType.mult, mybir.AluOpType.add,
                        )
                        nc.scalar.activation(
                            ut[:], ut[:], AF.Prelu,
                            bias=b_col[:], scale=1.0, alpha=acol_sb[:],
                        )
                        nc.vector.tensor_scalar(
                            ut[:], ut[:], CLAMP, -CLAMP,
                            mybir.AluOpType.min, mybir.AluOpType.max,
                        )
                        nc.sync.dma_start(y[:, off : off + NCHUNK], ut[:])

    nc.compile()
    return nc


def _get_nc():
    global _NC_CACHE
    if _NC_CACHE is None:
        _NC_CACHE = build_nc()
    return _NC_CACHE


def _make_core_inputs(x, w, affine_weight, affine_bias, weight, noise_const,
                      noise_strength, bias):
    """Build the 8 per-core input maps (host-side sharding / layout only)."""
    aff_host = np.ascontiguousarray(
        affine_weight.T.reshape(4, P, P).transpose(1, 0, 2)
    )  # [wd_p, j, ci]
    wt_host = np.ascontiguousarray(
        weight.reshape(P, P, TAPS).transpose(1, 2, 0)
    )  # [ci, k, co]
    ab_host = affine_bias.reshape(P, 1).astype(np.float32)
    bb_host = bias.reshape(P, 1).astype(np.float32)
    nsb_host = np.full((P, 1), float(noise_strength.reshape(-1)[0]), np.float32)
    ones_host = np.ones((P, 2), np.float32)
    acol_host = np.full((P, 1), LRELU_ALPHA, np.float32)
    epsc_host = np.full((P, 1), 1e-8, np.float32)
    zc_host = np.zeros((P, 1), np.float32)

    in_maps = []
    for c in range(8):
        b, half = divmod(c, 2)
        d0 = DHALF * half
        slab = np.zeros((P, NSLAB), np.float32)
        view = slab[:, LEAD : LEAD + BODY].reshape(P, NSLICES, ROW, ROW)
        lo = max(0, d0 - 1)
        hi = min(RES, d0 + DHALF + 1)
        # padded slice s holds global slice d0-1+s
        view[:, lo - (d0 - 1) : hi - (d0 - 1), :RES, :RES] = x[b, :, lo:hi]
        slab_hi = slab.astype(E4)
        slab_lo = (slab - slab_hi.astype(np.float32)).astype(E4)
        nz_host = np.ascontiguousarray(
            noise_const[d0 : d0 + DHALF].reshape(1, NOUT)
        )
        wv_host = np.zeros((P, 5), np.float32)
        wv_host[:, :4] = w[b].reshape(4, P).T
        in_maps.append({
            "xhi": slab_hi,
            "xlo": slab_lo,
            "wt": wt_host,
            "aff": aff_host,
            "wv": wv_host,
            "ab": ab_host,
            "bb": bb_host,
            "nz": nz_host,
            "nsb": nsb_host,
            "ones": ones_host,
            "acol": acol_host,
            "epsc": epsc_host,
            "zc": zc_host,
        })
    return in_maps


def kernel(x, w, affine_weight, affine_bias, weight, noise_const,
           noise_strength, bias):
    global LAST_EXEC_NS
    x = np.asarray(x, np.float32)
    w = np.asarray(w, np.float32)
    affine_weight = np.asarray(affine_weight, np.float32)
    affine_bias = np.asarray(affine_bias, np.float32)
    weight = np.asarray(weight, np.float32)
    noise_const = np.asarray(noise_const, np.float32)
    noise_strength = np.asarray(noise_strength, np.float32)
    bias = np.asarray(bias, np.float32)

    nc = _get_nc()
    in_maps = _make_core_inputs(
        x, w, affine_weight, affine_bias, weight, noise_const,
        noise_strength, bias,
    )
    trace = bool(os.environ.get("KERNEL_TRACE"))
    if trace:
        from concourse.bass_utils import axon_active

        if axon_active():
            try:  # axon NTFF capture needs the profile hook; absent in some pods
                from antenv.axon_hooks import get_axon_ntff_profile_hook  # noqa: F401
            except ImportError:
                trace = False
    res = run_bass_kernel_spmd(nc, in_maps, core_ids=list(range(8)), trace=trace)
    LAST_EXEC_NS = res.exec_time_ns

    out = np.empty((B, P, RES, RES, RES), np.float32)
    for c in range(8):
        b, half = divmod(c, 2)
        d0 = DHALF * half
        out[b, :, d0 : d0 + DHALF] = res.results[c]["y"].reshape(
            P, DHALF, RES, RES
        )
    return out


# revision 5
# speedup vs baseline: 1.3687x; 1.0619x over previous
"""TRN2 Bass kernel for nn_SynthesisLayer (StyleGAN-style modulated 3D conv).

Math: for each sample b
  styles = w[b] @ affine_weight.T / sqrt(512) + affine_bias          [Cin]
  wmod   = weight * styles[None,:,None]                              [Co,Ci,27]
  dcoef  = rsqrt(sum_{ci,k} wmod^2 + 1e-8)                           [Co]
  y      = dcoef * conv3d(x[b], wmod, pad=1) + noise_const*ns + bias
  out    = clip(lrelu(y)*sqrt(2), -256, 256)

Device implementation (per core):
  - conv3d = shifted matmuls (contraction over Cin=128 on partitions)
    accumulated in PSUM, weights modulated on device by styles.
  - fp8 DoubleRow path: x is pre-split on the host into e4m3 hi+lo slabs;
    modulated (bf16-shipped) weights are split on device into e4m3 hi+lo.
    Each DoubleRow matmul computes two (weight, shifted-x) products per
    PSUM row at 0.5 cycles/row.  Per output element: 27 hi*hi products +
    27 w_hi*x_lo + 22 w_lo*x_hi corrections (w_lo of taps 22-26 dropped,
    rel err ~1.3% vs the 2e-2 budget) = 76 products = 38 matmuls per
    512-wide tile, vs 27 full-rate fp32r matmuls for the exact conv.
  - demod + noise fold into a per-partition scale (ACT Prelu) and a DVE
    scalar_tensor_tensor; demod squares/reductions run on the Pool engine
    off the DVE critical path.

Sharding: 8 cores = 4 samples x 2 D-halves. Each core gets zero-padded
fp8 hi/lo input slabs [128, NSLAB] (33-wide rows, 33-row slices, one-slice
D halo), computes output [128, 16*32*32], host reassembles. No collectives.
"""

import math
import os
import sys

for _p in ("/opt/trn_rl_repo", "/root/.axon_site/_ro/trn_rl_repo"):
    if os.path.isdir(_p) and _p not in sys.path:
        sys.path.insert(0, _p)

import numpy as np
import ml_dtypes

import concourse.mybir as mybir
from concourse import bacc
from concourse.ap import AP
from concourse.tile import TileContext
from concourse.bass_utils import run_bass_kernel_spmd

P = 128          # Cin = Cout = 128
TAPS = 27        # 3x3x3
NDROP = 5        # taps whose w_lo correction is dropped (22..26)
NLO = TAPS - NDROP
RES = 32
B = 4
W_DIM = 512
ROW = 33         # padded row width  (32 real + 1 zero)
SLICE = ROW * ROW  # 1089 padded slice (32 real rows + 1 zero row)
LEAD = 34        # leading zero guard (one row + one elem)
NSLICES = 18     # 16 output slices + 1 halo each side
BODY = NSLICES * SLICE
NSLAB = LEAD + BODY + 46   # tail guard; max AP end = 19637
DHALF = 16                 # output D slices per core
NOUT = DHALF * RES * RES   # 16384
DCH = 4                    # output D slices per pipelined input chunk
NSLABC = LEAD + (DCH + 2) * SLICE + 46  # 6614: chunk tile incl. halo+guards
NCHUNK = 512               # psum tile free size (one PSUM bank of fp32)
LRELU_ALPHA = 0.2
LRELU_GAIN = math.sqrt(2.0)
CLAMP = 256.0

f32 = mybir.dt.float32
f32r = mybir.dt.float32r
bf16 = mybir.dt.bfloat16
fp8 = mybir.dt.float8e4
DRMODE = mybir.MatmulPerfMode.DoubleRow
AF = mybir.ActivationFunctionType
E4 = ml_dtypes.float8_e4m3fn

# tap k = kd*9 + kh*3 + kw; shift of tap k relative to the tile center
TAP_OFF = [
    (kd - 1) * SLICE + (kh - 1) * ROW + (kw - 1)
    for kd in range(3) for kh in range(3) for kw in range(3)
]

_NC_CACHE = None
LAST_EXEC_NS = None


def _pair_ap(flat_ap, off, delta, inner_dims):
    """[[p],[delta,2],*inner_dims] AP at element offset `off` of a 2D AP."""
    dims = [list(flat_ap.ap[0]), [delta, 2]] + [list(d) for d in inner_dims]
    return AP(flat_ap.tensor, flat_ap.offset + off, dims)


def build_nc():
    nc = bacc.Bacc("TRN2", target_bir_lowering=False, debug=False, num_devices=8)
    pool = nc.engines[mybir.EngineType.Pool]

    xhi = nc.dram_tensor("xhi", [P, NSLAB], fp8, kind="ExternalInput")
    xlo = nc.dram_tensor("xlo", [P, NSLAB], fp8, kind="ExternalInput")
    wt = nc.dram_tensor("wt", [P, TAPS, P], bf16, kind="ExternalInput")
    aff = nc.dram_tensor("aff", [P, 4, P], bf16, kind="ExternalInput")
    wv = nc.dram_tensor("wv", [P, 5], bf16, kind="ExternalInput")
    ab = nc.dram_tensor("ab", [P, 1], f32, kind="ExternalInput")
    bb = nc.dram_tensor("bb", [P, 1], f32, kind="ExternalInput")
    nz = nc.dram_tensor("nz", [1, NOUT], f32, kind="ExternalInput")
    nsb = nc.dram_tensor("nsb", [P, 1], f32, kind="ExternalInput")
    ones = nc.dram_tensor("ones", [P, 2], f32r, kind="ExternalInput")
    acol = nc.dram_tensor("acol", [P, 1], f32, kind="ExternalInput")
    epsc = nc.dram_tensor("epsc", [P, 1], f32, kind="ExternalInput")
    zc = nc.dram_tensor("zc", [P, 1], f32, kind="ExternalInput")
    y = nc.dram_tensor("y", [P, NOUT], f32, kind="ExternalOutput")

    with TileContext(nc) as tc:
        with (
            tc.tile_pool(name="big", bufs=1) as big,
            tc.tile_pool(name="small", bufs=1) as small,
            tc.tile_pool(name="nzp", bufs=4) as nzp,
            tc.tile_pool(name="xchunk", bufs=2) as xchunk,
            tc.tile_pool(name="outp", bufs=4) as outp,
            tc.tile_pool(name="cpsum", bufs=6, space="PSUM") as cpsum,
            tc.tile_pool(name="spsum", bufs=1, space="PSUM") as spsum,
        ):
            # ---- small loads first: the style path comes off these ----
            aff_sb = small.tile([P, 4, P], bf16)
            nc.sync.dma_start(aff_sb[:], aff[:])
            wv_sb = small.tile([P, 5], bf16)
            nc.sync.dma_start(wv_sb[:], wv[:])
            ab_sb = small.tile([P, 1], f32)
            nc.sync.dma_start(ab_sb[:], ab[:])
            bb_sb = small.tile([P, 1], f32)
            nc.sync.dma_start(bb_sb[:], bb[:])
            nsb_sb = small.tile([P, 1], f32)
            nc.sync.dma_start(nsb_sb[:], nsb[:])
            ones_sb = small.tile([P, 2], f32r)
            nc.sync.dma_start(ones_sb[:], ones[:])
            acol_sb = small.tile([P, 1], f32)
            nc.sync.dma_start(acol_sb[:], acol[:])
            epsc_sb = small.tile([P, 1], f32)
            nc.sync.dma_start(epsc_sb[:], epsc[:])
            zc_sb = small.tile([P, 1], f32)
            nc.sync.dma_start(zc_sb[:], zc[:])

            # ---- x chunk 1 (hi first: the PE main pass needs only hi),
            # then the weights, then x chunk 1 lo ----
            CHUNKS = [(1, 1), (2, 3), (5, 4), (9, 4), (13, 4)]  # (a, n_out)
            xt1 = xchunk.tile([P, 2, NSLABC], fp8, tag="xchunk")
            wlen1 = LEAD + 3 * SLICE + 46
            step = 2 * SLICE
            for i0 in range(0, wlen1, step):
                bnd = min(wlen1, i0 + step)
                nc.sync.dma_start(xt1[:, 1, i0:bnd], xhi[:, i0:bnd])

            # weight DMA in tap chunks so quantization starts early
            wt_sb = big.tile([P, TAPS, P], bf16)
            WCH = [(0, 7), (7, 14), (14, 21), (21, TAPS)]
            for c0, c1 in WCH:
                nc.sync.dma_start(wt_sb[:, c0:c1, :], wt[:, c0:c1, :])

            for i0 in range(0, wlen1, step):
                bnd = min(wlen1, i0 + step)
                nc.sync.dma_start(xt1[:, 0, i0:bnd], xlo[:, i0:bnd])

            # ---- styles = w[b] @ aff.T / sqrt(512) + affine_bias ----
            # N=1 matmuls are ISA-illegal; use N=2 and read col 0
            st_ps = spsum.tile([P, 2], f32, tag="st")
            for j in range(4):
                nc.tensor.matmul(
                    st_ps[:], aff_sb[:, j, :], wv_sb[:, j : j + 2],
                    start=(j == 0), stop=(j == 3),
                )
            styles = small.tile([P, 1], f32)
            nc.scalar.activation(
                styles[:], st_ps[:, :1], AF.Identity,
                bias=ab_sb[:], scale=1.0 / math.sqrt(W_DIM),
            )

            # ---- modulated weights, split into fp8 hi + lo ----
            # wq slots 0..26 = hi taps, 27..48 = lo taps 0..21
            wq = big.tile([P, TAPS + NLO, P], fp8)
            for c0, c1 in WCH:
                nc.vector.tensor_scalar_mul(
                    wq[:, c0:c1, :], wt_sb[:, c0:c1, :], styles[:]
                )
            for c0, c1 in ((0, 11), (11, NLO)):
                nc.vector.scalar_tensor_tensor(
                    wq[:, TAPS + c0 : TAPS + c1, :], wt_sb[:, c0:c1, :],
                    styles[:], wq[:, c0:c1, :],
                    mybir.AluOpType.mult, mybir.AluOpType.subtract,
                )
            wq_flat = wq[:].rearrange("p a b -> p (a b)")

            # ---- demod sums: v[co] = sum_ci styles^2 * (sum_k wt^2) ----
            # on the Pool engine, off the DVE weight-prep critical path
            sq_sb = big.tile([P, P, TAPS], f32)  # [ci, co, k]
            pool.tensor_tensor(
                sq_sb.rearrange("p co k -> p k co"), wt_sb[:], wt_sb[:],
                mybir.AluOpType.mult,
            )
            wsq_f = big.tile([P, P], f32)  # [ci, co] = sum_k wt^2
            nc.vector.reduce_sum(wsq_f[:], sq_sb[:], axis=mybir.AxisListType.X)
            s2 = small.tile([P, 1], f32)
            pool.tensor_tensor(s2[:], styles[:], styles[:], mybir.AluOpType.mult)
            w2s_f = big.tile([P, P], f32)
            pool.tensor_scalar_mul(w2s_f[:], wsq_f[:], s2[:])
            w2s = big.tile([P, P], f32r)
            pool.tensor_copy(w2s[:], w2s_f[:])

            vcol_ps = spsum.tile([P, 2], f32, tag="vc")
            nc.tensor.matmul(vcol_ps[:], w2s[:], ones_sb[:], start=True, stop=True)

            # S_col = sqrt(2) * rsqrt(v + 1e-8)  (per-partition ACT scale)
            veps = small.tile([P, 1], f32)
            nc.scalar.activation(veps[:], vcol_ps[:, :1], AF.Identity, bias=epsc_sb[:])
            vrec = small.tile([P, 1], f32)
            nc.vector.reciprocal(vrec[:], veps[:])
            s_col = small.tile([P, 1], f32)
            nc.scalar.activation(
                s_col[:], vrec[:], AF.Sqrt, bias=zc_sb[:], scale=LRELU_GAIN**2
            )
            # B_col = bias * sqrt(2)
            b_col = small.tile([P, 1], f32)
            nc.vector.tensor_scalar_mul(b_col[:], bb_sb[:], LRELU_GAIN)
            # noise gain = noise_strength * sqrt(2), per partition
            nsg = small.tile([P, 1], f32)
            nc.vector.tensor_scalar_mul(nsg[:], nsb_sb[:], LRELU_GAIN)

            # ---- main conv loop: variable input chunks (double-buffered);
            # the first chunk is small so PE starts sooner ----
            def conv_tile(xt_flat, n0, off, width):
                """One PSUM accumulation group + epilogue for `width` outputs
                centred at padded offset n0, writing y[off:off+width]."""
                nrows = width // RES
                inner = ([ROW, nrows], [1, RES])
                # noise first: no PSUM dependency, so the epilogue can fire
                # the moment the accumulation group closes
                nz_bc = nzp.tile([P, 1, width], f32, tag="nz")
                nc.sync.dma_start(
                    nz_bc[:], nz[:, off : off + width].partition_broadcast(P)
                )
                pool.tensor_scalar_mul(nz_bc[:], nz_bc[:], nsg[:])

                pt = cpsum.tile([P, width], f32, tag="conv")
                # 13 hi-hi tap pairs, tap26 hi*(lo+hi), 2 x_lo pairs for taps
                # 22-25 (all need only w_hi), then 22 hi/lo correction pairs
                # (need w_lo, off the startup critical path)
                for i in range(13):
                    q0 = n0 + TAP_OFF[2 * i]
                    dq = TAP_OFF[2 * i + 1] - TAP_OFF[2 * i]
                    nc.tensor.matmul(
                        pt[:], wq[:, 2 * i : 2 * i + 2, :],
                        _pair_ap(xt_flat, NSLABC + q0, dq, inner),
                        start=(i == 0), stop=False, perf_mode=DRMODE,
                    )
                nc.tensor.matmul(
                    pt[:], _pair_ap(wq_flat, 26 * P, 0, ([1, P],)),
                    _pair_ap(xt_flat, n0 + TAP_OFF[26], NSLABC, inner),
                    start=False, stop=False, perf_mode=DRMODE,
                )
                for k in (22, 24):
                    qk = n0 + TAP_OFF[k]
                    dq = TAP_OFF[k + 1] - TAP_OFF[k]
                    nc.tensor.matmul(
                        pt[:], wq[:, k : k + 2, :],
                        _pair_ap(xt_flat, qk, dq, inner),
                        start=False, stop=False, perf_mode=DRMODE,
                    )
                for k in range(NLO):
                    qk = n0 + TAP_OFF[k]
                    nc.tensor.matmul(
                        pt[:], _pair_ap(wq_flat, k * P, TAPS * P, ([1, P],)),
                        _pair_ap(xt_flat, qk, NSLABC, inner),
                        start=False, stop=(k == NLO - 1), perf_mode=DRMODE,
                    )

                ut = outp.tile([P, width], f32, tag="out")
                # ut = psum * (dcoef*sqrt2) + noise_term
                nc.vector.scalar_tensor_tensor(
                    ut[:], pt[:], s_col[:], nz_bc[:, 0, :],
                    mybir.AluOpType.mult, mybir.AluOpType.add,
                )
                nc.scalar.activation(
                    ut[:], ut[:], AF.Prelu,
                    bias=b_col[:], scale=1.0, alpha=acol_sb[:],
                )
                nc.vector.tensor_scalar(
                    ut[:], ut[:], CLAMP, -CLAMP,
                    mybir.AluOpType.min, mybir.AluOpType.max,
                )
                nc.sync.dma_start(y[:, off : off + width], ut[:])

            for ci, (a, n) in enumerate(CHUNKS):
                last_chunk = ci == len(CHUNKS) - 1
                if ci == 0:
                    xt = xt1  # chunk 1 was DMA'd during startup
                else:
                    xt = xchunk.tile([P, 2, NSLABC], fp8, tag="xchunk")
                    s_g = (a - 1) * SLICE  # chunk start in the padded slab
                    wlen = LEAD + (n + 2) * SLICE + 46
                    for sl in (1, 0):
                        for i0 in range(0, wlen, step):
                            bnd = min(wlen, i0 + step)
                            nc.sync.dma_start(
                                xt[:, sl, i0:bnd],
                                (xhi if sl else xlo)[:, s_g + i0 : s_g + bnd],
                            )
                xt_flat = xt[:].rearrange("p a b -> p (a b)")
                for dl in range(1, n + 1):       # local padded slice index
                    d = a + dl - 1               # global padded slice index
                    for half in range(2):        # 16 rows each
                        n0 = LEAD + dl * SLICE + half * 16 * ROW
                        off = (d - 1) * 1024 + half * NCHUNK
                        if last_chunk and dl == n and half == 1:
                            # split the final tile so the tail drain is short
                            conv_tile(xt_flat, n0, off, NCHUNK // 2)
                            conv_tile(
                                xt_flat, n0 + 8 * ROW, off + NCHUNK // 2,
                                NCHUNK // 2,
                            )
                        else:
                            conv_tile(xt_flat, n0, off, NCHUNK)

    nc.compile()
    return nc


def _get_nc():
    global _NC_CACHE
    if _NC_CACHE is None:
        _NC_CACHE = build_nc()
    return _NC_CACHE


def _make_core_inputs(x, w, affine_weight, affine_bias, weight, noise_const,
                      noise_strength, bias):
    """Build the 8 per-core input maps (host-side sharding / layout only)."""
    aff_host = np.ascontiguousarray(
        affine_weight.T.reshape(4, P, P).transpose(1, 0, 2)
    ).astype(ml_dtypes.bfloat16)  # [wd_p, j, ci]
    wt_host = np.ascontiguousarray(
        weight.reshape(P, P, TAPS).transpose(1, 2, 0)
    ).astype(ml_dtypes.bfloat16)  # [ci, k, co]
    ab_host = affine_bias.reshape(P, 1).astype(np.float32)
    bb_host = bias.reshape(P, 1).astype(np.float32)
    nsb_host = np.full((P, 1), float(noise_strength.reshape(-1)[0]), np.float32)
    ones_host = np.ones((P, 2), np.float32)
    acol_host = np.full((P, 1), LRELU_ALPHA, np.float32)
    epsc_host = np.full((P, 1), 1e-8, np.float32)
    zc_host = np.zeros((P, 1), np.float32)

    in_maps = []
    for c in range(8):
        b, half = divmod(c, 2)
        d0 = DHALF * half
        slab = np.zeros((P, NSLAB), np.float32)
        view = slab[:, LEAD : LEAD + BODY].reshape(P, NSLICES, ROW, ROW)
        lo = max(0, d0 - 1)
        hi = min(RES, d0 + DHALF + 1)
        # padded slice s holds global slice d0-1+s
        view[:, lo - (d0 - 1) : hi - (d0 - 1), :RES, :RES] = x[b, :, lo:hi]
        slab_hi = slab.astype(E4)
        slab_lo = (slab - slab_hi.astype(np.float32)).astype(E4)
        nz_host = np.ascontiguousarray(
            noise_const[d0 : d0 + DHALF].reshape(1, NOUT)
        )
        wv_host = np.zeros((P, 5), np.float32)
        wv_host[:, :4] = w[b].reshape(4, P).T
        in_maps.append({
            "xhi": slab_hi,
            "xlo": slab_lo,
            "wt": wt_host,
            "aff": aff_host,
            "wv": wv_host.astype(ml_dtypes.bfloat16),
            "ab": ab_host,
            "bb": bb_host,
            "nz": nz_host,
            "nsb": nsb_host,
            "ones": ones_host,
            "acol": acol_host,
            "epsc": epsc_host,
            "zc": zc_host,
        })
    return in_maps


def kernel(x, w, affine_weight, affine_bias, weight, noise_const,
           noise_strength, bias):
    global LAST_EXEC_NS
    x = np.asarray(x, np.float32)
    w = np.asarray(w, np.float32)
    affine_weight = np.asarray(affine_weight, np.float32)
    affine_bias = np.asarray(affine_bias, np.float32)
    weight = np.asarray(weight, np.float32)
    noise_const = np.asarray(noise_const, np.float32)
    noise_strength = np.asarray(noise_strength, np.float32)
    bias = np.asarray(bias, np.float32)

    nc = _get_nc()
    in_maps = _make_core_inputs(
        x, w, affine_weight, affine_bias, weight, noise_const,
        noise_strength, bias,
    )
    trace = bool(os.environ.get("KERNEL_TRACE"))
    if trace:
        from concourse.bass_utils import axon_active

        if axon_active():
            try:  # axon NTFF capture needs the profile hook; absent in some pods
                from antenv.axon_hooks import get_axon_ntff_profile_hook  # noqa: F401
            except ImportError:
                trace = False
    res = run_bass_kernel_spmd(nc, in_maps, core_ids=list(range(8)), trace=trace)
    LAST_EXEC_NS = res.exec_time_ns

    out = np.empty((B, P, RES, RES, RES), np.float32)
    for c in range(8):
        b, half = divmod(c, 2)
        d0 = DHALF * half
        out[b, :, d0 : d0 + DHALF] = res.results[c]["y"].reshape(
            P, DHALF, RES, RES
        )
    return out
